# revision 47
# baseline (speedup 1.0000x reference)
"""Trainium2 Bass kernel for nn_AttentionBlock (GroupNorm -> 1x1 qkv -> full
N^2 attention -> 1x1 proj -> residual) on x:(4, 512, 64, 64).

Sharding: 8 cores = (batch, query-half) pairs. Each core gets one batch's
full image (512 x 4096 pixels) with pixels rotated so that its query half is
always pixels [0:2048]; softmax/attention are permutation-invariant in the
key axis, so every core runs the identical SPMD graph with no collectives.

GroupNorm is folded into the qkv matmul: xn = sc*x + bs per channel, so
qkv = (W*diag(sc)) x + (b + W bs). The host ships x as fp8 (0.5*x) next to
the bf16 stats copy; sc rides the existing bf16->fp8 weight cast as a
per-partition activation scale, and the bias correction W bs comes from two
tiny DoubleRow matvecs (bs/sc column for q/k blocks; a 1-row x W product,
partition-broadcast by a ones matmul, for V^T). This takes the 16K-elem xn
pass off the critical path and lets qkv start right after the group stats.

Attention runs transpose-free in a key-on-partitions layout: S^T = K^T Q per
128-key block (contraction over channels), exp'd in place to fp8 P^T tiles;
O = sum_j V^T^T P^T accumulates over key blocks, so O lands as [c-part,
i-free] -- exactly what proj wants. The softmax denominator rides a
128-identical-columns ones DoubleRow matmul (l replicated across all
partitions), and 1/l comes from one fast-approx DVE reciprocal, folded into
the PSUM->SBUF eviction of O. proj psums triple-buffer through the S^T
PSUM pool and each segment's proj is interleaved into the next segment's
tensor stream so evictions never stall the queue.

All big matmuls run fp8e4 DoubleRow with fp32 PSUM. Scaling: x8 = 0.5*x,
w8 = 8*sc*w, q8/k8 = 4*c^-0.25 * (q/k), vt = 4*V^T, P8 = exp(S - 2.5)
(no max subtraction; |S| <= ~8), o8 = 4*O. The group stats run directly on
the fp8 x (split VectorE bn_stats / ScalarE accum across channel blocks;
two HWDGE queues split the input stream), and the group reduce/broadcast
is two tiny indicator matmuls instead of transposes.

HW-measured: 228.0 us per NEFF (baseline 276.9 us), rel err 1.8e-3.
"""

import os
import numpy as np

C = 512
CB = 4            # 128-channel blocks
N = 4096          # pixels per image
NH = 2048         # query pixels per core
G = 32            # groups
EPS = 1e-6
SCALE = float(C) ** -0.25
FD = 512          # psum free width
NSEG = NH // FD   # query segments per core (4)
JB = N // 128     # key blocks (32)

_CACHE = {}


def build_bass():
    import concourse.bass as bass
    import concourse.mybir as mybir
    import concourse.tile as tile
    from concourse import bacc
    from concourse.bass import ts
    f32 = mybir.dt.float32
    bf16 = mybir.dt.bfloat16
    fp8 = mybir.dt.float8e4
    AF = mybir.ActivationFunctionType
    ALU = mybir.AluOpType
    AX = mybir.AxisListType
    DR = mybir.MatmulPerfMode.DoubleRow

    nc = bacc.Bacc(None)
    x8_ext = nc.declare_dram_parameter("x8", [C, N], fp8, isOutput=False)
    gind_ext = nc.declare_dram_parameter("gind", [128, 8], f32, isOutput=False)
    gindT_ext = nc.declare_dram_parameter("gindT", [8, 128], f32, isOutput=False)
    bqs_ext = nc.declare_dram_parameter("bqs", [1024], f32, isOutput=False)
    bv4_ext = nc.declare_dram_parameter("bv4", [C], f32, isOutput=False)
    # note: bqkv/bproj reach the device only in folded form (bqs, bv4, xres)
    xres_ext = nc.declare_dram_parameter("xres", [C, NH], f32, isOutput=False)
    gamma_ext = nc.declare_dram_parameter("gamma", [C], f32, isOutput=False)
    beta_ext = nc.declare_dram_parameter("beta", [C], f32, isOutput=False)
    wqkvT_ext = nc.declare_dram_parameter("wqkvT", [C, 3 * C], bf16, isOutput=False)
    wprojT_ext = nc.declare_dram_parameter("wprojT", [C, C], bf16, isOutput=False)
    out_ext = nc.declare_dram_parameter("out", [C, NH], f32, isOutput=True)

    with tile.TileContext(nc) as tc:
        with (
            tc.tile_pool(name="const", bufs=1) as cpool,
            tc.tile_pool(name="big", bufs=1) as bigpool,
        ):
            # pools entered before xphase so they outlive it (LIFO release)
            p8phase = tc.tile_pool(name="p8", bufs=2)
            ppool = p8phase.__enter__()
            sphase = tc.tile_pool(name="spsum", bufs=3, space="PSUM")
            spool = sphase.__enter__()

            # x streams in as fp8 only (group stats run on it too -- the
            # quantization noise washes out over 64K elements/group). The two
            # HWDGE queues split the load: sync takes VectorE's stats blocks
            # (0,1) + qkv weights, scalar takes ScalarE's blocks (3,2) + wp.
            xphase = tc.tile_pool(name="xph", bufs=1)
            xpool = xphase.__enter__()
            x8 = xpool.tile([128, CB, N], fp8)
            for cc, hh in ((0, 0), (0, 1), (1, 0), (1, 1)):
                nc.sync.dma_start(
                    out=x8[:, cc, ts(hh, NH)],
                    in_=x8_ext[cc * 128:(cc + 1) * 128, ts(hh, NH)],
                )
            for cc, hh in ((3, 0), (3, 1), (2, 0), (2, 1)):
                nc.scalar.dma_start(
                    out=x8[:, cc, ts(hh, NH)],
                    in_=x8_ext[cc * 128:(cc + 1) * 128, ts(hh, NH)],
                )

            # ---- constants / weights ----
            # group indicator matmul operands: gind[p,g]=1/(16N) for
            # p//16==g (group-sum as one matmul), gindT[g,p]=1 indicator
            # (partition-broadcast of group values as one matmul)
            # small consts ride the gpsimd software queue so the two HW
            # queues stay clear for x8/weights (the rearranged bias loads
            # are descriptor-heavy)
            gind_sb = cpool.tile([128, 8], f32)
            nc.gpsimd.dma_start(out=gind_sb, in_=gind_ext[:, :])
            gindT_sb = cpool.tile([8, 128], f32)
            nc.gpsimd.dma_start(out=gindT_sb, in_=gindT_ext[:, :])

            gb_sb = cpool.tile([128, 2, CB], f32)  # gamma, beta as (p, t)
            nc.gpsimd.dma_start(out=gb_sb[:, 0, :], in_=gamma_ext.rearrange("(t p) -> p t", p=128))
            nc.gpsimd.dma_start(out=gb_sb[:, 1, :], in_=beta_ext.rearrange("(t p) -> p t", p=128))

            # q,k bias blocks pre-scaled by 4*SCALE on the host
            bqs_sb = cpool.tile([128, 8], f32)
            nc.gpsimd.dma_start(out=bqs_sb, in_=bqs_ext.rearrange("(t p) -> p t", p=128))

            # (b_proj is folded into xres on the host)
            # 4*b_v (host-scaled) broadcast along partitions: (128, 512)
            bvt_sb = cpool.tile([128, FD], f32)
            bv_slice = bv4_ext[:]
            bv_bcast = bass.AP(
                tensor=bv_slice.tensor,
                offset=bv_slice.offset,
                ap=[[0, 128]] + [list(p) for p in bv_slice.ap],
            )
            nc.gpsimd.dma_start(out=bvt_sb, in_=bv_bcast)

            eps_sb = cpool.tile([128, 1], f32)
            nc.vector.memset(eps_sb, EPS)
            nbias_sb = cpool.tile([128, 1], f32)  # global exp bias
            nc.vector.memset(nbias_sb, -2.5)
            warm_sb = cpool.tile([128, 1], f32)
            # DR all-ones stationary, 128 identical columns -> l-sum lands on
            # every partition (no separate broadcast needed)
            ones128 = cpool.tile([128, 2, 128], fp8)
            nc.vector.memset(ones128, 1.0)
            ones32 = cpool.tile([1, 128], f32)    # 1-row ones for V-bias bcast
            nc.vector.memset(ones32, 1.0)

            wqbf = cpool.tile([128, CB, 3 * C], bf16)
            nc.sync.dma_start(out=wqbf, in_=wqkvT_ext.rearrange("(t p) o -> p t o", p=128))
            wq8 = cpool.tile([128, CB, 3 * C], fp8)   # 8*sc*W, cast after stats
            wp_sb = cpool.tile([128, CB, C], bf16)
            nc.scalar.dma_start(out=wp_sb, in_=wprojT_ext.rearrange("(t p) o -> p t o", p=128))
            wp8 = cpool.tile([128, CB, C], fp8)

            # ---- persistent activations ----
            k8_sb = bigpool.tile([128, CB, N], fp8)
            vt_sb = bigpool.tile([128, JB, FD], fp8)   # 4*V^T
            q8_sb = bigpool.tile([128, CB, NH], fp8)

            # ===== phase 1: groupnorm stats on fp8 x (= 0.5*x). stat2 keeps
            # the raw fp8 moments (mean8, meansq8); the x2/x4 rescale folds
            # into the tiny group-level math. VectorE: blocks 0,1,2 via
            # bn_stats; ScalarE: block 3 via Identity/Square accum. =====
            with tc.tile_pool(name="pst", bufs=2, space="PSUM") as pst:
                stat2 = xpool.tile([128, CB, 2], f32)  # (mean8, meansq8) per channel
                st_stats = xpool.tile([128, 3, 8, 6], f32)
                mv_t = xpool.tile([128, 3, 2], f32)
                sc_scratch = xpool.tile([128, 2048], bf16)
                acc_part = xpool.tile([128, 2, 2], f32)  # block 3 halves
                for s in range(2):
                    nc.scalar.activation(
                        out=sc_scratch, in_=x8[:, 3, ts(s, 2048)],
                        func=AF.Identity, bias=0.0, scale=1.0,
                        accum_out=acc_part[:, 0, s:s + 1],
                    )
                    nc.scalar.activation(
                        out=sc_scratch, in_=x8[:, 3, ts(s, 2048)],
                        func=AF.Square, bias=0.0, scale=1.0,
                        accum_out=acc_part[:, 1, s:s + 1],
                    )
                # warm the Sqrt table now; the rstd sqrt hits it without a load
                nc.scalar.activation(out=warm_sb, in_=eps_sb, func=AF.Sqrt, bias=0.0, scale=1.0)
                for cc in range(3):
                    for s in range(8):
                        nc.vector.bn_stats(out=st_stats[:, cc, s, :], in_=x8[:, cc, ts(s, 512)])
                    nc.vector.bn_aggr(out=mv_t[:, cc, :], in_=st_stats[:, cc])
                    # mean8 ; meansq8 = var8 + mean8^2
                    nc.vector.tensor_copy(stat2[:, cc, 0:1], mv_t[:, cc, 0:1])
                    nc.vector.tensor_mul(stat2[:, cc, 1:2], mv_t[:, cc, 0:1], mv_t[:, cc, 0:1])
                    nc.vector.tensor_add(stat2[:, cc, 1:2], stat2[:, cc, 1:2], mv_t[:, cc, 1:2])
                for f in range(2):
                    nc.vector.tensor_reduce(
                        out=stat2[:, 3, f:f + 1], in_=acc_part[:, f, :],
                        axis=AX.X, op=ALU.add,
                    )
                    nc.vector.tensor_scalar_mul(stat2[:, 3, f:f + 1], stat2[:, 3, f:f + 1], 1.0 / float(N))

                # group aggregation as two tiny matmuls: gind^T @ stat2 =
                # per-group fp8 moments; gindT^T @ vals broadcasts the
                # (mean_x, rstd) pair back to every channel partition
                gs_ps = pst.tile([8, 8], f32)
                nc.tensor.matmul(gs_ps, lhsT=gind_sb, rhs=stat2[:, :, :])
                vals = xpool.tile([8, 2, CB], f32)  # (g, {mean_x, rstd}, cc)
                gsv = gs_ps.rearrange("g (cc f) -> g cc f", f=2)
                var_g = xpool.tile([8, CB], f32)
                nc.vector.tensor_scalar_mul(vals[:, 0, :], gsv[:, :, 0], 2.0)
                nc.vector.tensor_scalar_mul(var_g, gsv[:, :, 1], 4.0)
                nc.vector.tensor_mul(vals[:, 1, :], vals[:, 0, :], vals[:, 0, :])
                nc.vector.tensor_tensor(var_g, var_g, vals[:, 1, :], ALU.subtract)
                nc.scalar.activation(out=var_g, in_=var_g, func=AF.Sqrt, bias=eps_sb[:8], scale=1.0)
                nc.vector.reciprocal(vals[:, 1, :], var_g)
                mr_ps = pst.tile([128, 8], f32)
                nc.tensor.matmul(mr_ps, lhsT=gindT_sb, rhs=vals[:, :, :])

                # per-channel xn = sc*x + bs; sc folds into the weight cast,
                # bs into bias-correction matvecs (bsc8 = 16*bs/sc column)
                sc_sb = xpool.tile([128, CB], f32)
                bs_sb = xpool.tile([128, CB], f32)
                tmp_c = xpool.tile([128, CB], f32)
                nc.vector.tensor_mul(sc_sb, gb_sb[:, 0, :], mr_ps[:, 4:8])
                nc.vector.tensor_mul(tmp_c, mr_ps[:, 0:4], sc_sb)
                nc.vector.tensor_tensor(bs_sb, gb_sb[:, 1, :], tmp_c, ALU.subtract)
                scx8 = xpool.tile([128, CB], f32)
                nc.vector.tensor_scalar_mul(scx8, sc_sb, 8.0)
                rsc = xpool.tile([128, CB], f32)
                nc.vector.reciprocal(rsc, sc_sb)
                bsc = xpool.tile([128, CB], f32)
                nc.vector.tensor_mul(bsc, bs_sb, rsc)
                bsc8 = xpool.tile([128, CB, 16], fp8)  # col 0; 16B DR pair step
                nc.vector.tensor_scalar_mul(bsc8[:, :, 0:1], bsc[:, :, None], 16.0)

                # W' = 8*sc*W cast bf16->fp8, q cols first so Q starts ASAP;
                # split scalar/vector by cc block
                for cols in range(3):
                    for cc in range(CB):
                        src = wqbf[:, cc, ts(cols, C)]
                        dst = wq8[:, cc, ts(cols, C)]
                        if cc % 2 == 0:
                            nc.scalar.activation(
                                out=dst, in_=src, func=AF.Copy, bias=0.0,
                                scale=scx8[:, cc:cc + 1],
                            )
                        else:
                            nc.vector.tensor_scalar_mul(dst, src, scx8[:, cc:cc + 1])
                # warm the Exp table now so the first attention exp doesn't
                # pay the table load (Copy casts don't touch the table)
                nc.scalar.activation(out=warm_sb, in_=eps_sb, func=AF.Exp, bias=0.0, scale=1.0)
                nc.scalar.activation(out=wp8, in_=wp_sb, func=AF.Copy, bias=0.0, scale=8.0)

            # ====== phase 2: qkv projections fused with iseg0 S^T+exp ======
            if True:
                p8_0 = ppool.tile([128, JB, FD], fp8, tag="p8")

                with tc.tile_pool(name="mmps", bufs=5, space="PSUM") as mmps:
                    # q/k bias corrections: bias_ps[:, blk] = 128*(W bs)[blk]
                    bias_ps = mmps.tile([128, FD], f32, tag="qkvps", name="qkvps")
                    bqs_new = xpool.tile([128, 8], f32)
                    for half in range(2):  # q blocks 0-3, k blocks 4-7
                        for b4 in range(CB):
                            blk = 4 * half + b4
                            for t in range(2):
                                nc.tensor.matmul(
                                    bias_ps[:, blk:blk + 1],
                                    lhsT=wq8[:, 2 * t:2 * t + 2, ts(blk, 128)],
                                    rhs=bsc8[:, 2 * t:2 * t + 2, 0:1],
                                    start=(t == 0), stop=(t == 1), perf_mode=DR,
                                )
                        nc.vector.scalar_tensor_tensor(
                            out=bqs_new[:, 4 * half:4 * half + 4],
                            in0=bias_ps[:, 4 * half:4 * half + 4],
                            scalar=SCALE / 32.0,
                            in1=bqs_sb[:, 4 * half:4 * half + 4],
                            op0=ALU.mult, op1=ALU.add,
                        )

                    for ob in range(CB):  # Q, first NH pixels
                        pss = [mmps.tile([128, FD], f32, tag="qkvps", name="qkvps") for _ in range(NSEG)]
                        for t in range(2):
                            for iseg in range(NSEG):
                                nc.tensor.matmul(
                                    pss[iseg],
                                    lhsT=wq8[:, 2 * t:2 * t + 2, ts(ob, 128)],
                                    rhs=x8[:, 2 * t:2 * t + 2, ts(iseg, FD)],
                                    start=(t == 0), stop=(t == 1), perf_mode=DR,
                                )
                        for iseg in range(NSEG):
                            nc.vector.tensor_scalar(
                                out=q8_sb[:, ob, ts(iseg, FD)], in0=pss[iseg],
                                scalar1=SCALE, scalar2=bqs_new[:, ob:ob + 1],
                                op0=ALU.mult, op1=ALU.add,
                            )

                    # V bias correction row: 128*(W_v bs) as [1, 512], then
                    # partition-broadcast via ones32 matmul, folded into bvt
                    ps_vr = mmps.tile([128, FD], f32, tag="qkvps", name="qkvps")
                    for t in range(2):
                        nc.tensor.matmul(
                            ps_vr[0:1, :],
                            lhsT=bsc8[:, 2 * t:2 * t + 2, 0:1],
                            rhs=wq8[:, 2 * t:2 * t + 2, 1024:1536],
                            start=(t == 0), stop=(t == 1), perf_mode=DR,
                        )
                    vrow_sb = xpool.tile([1, FD], f32)
                    nc.scalar.activation(out=vrow_sb, in_=ps_vr[0:1, :], func=AF.Copy, bias=0.0, scale=1.0)
                    ps_vb = mmps.tile([128, FD], f32, tag="qkvps", name="qkvps")
                    nc.tensor.matmul(ps_vb, lhsT=ones32, rhs=vrow_sb)
                    nc.vector.scalar_tensor_tensor(
                        out=bvt_sb, in0=ps_vb, scalar=1.0 / 32.0, in1=bvt_sb,
                        op0=ALU.mult, op1=ALU.add,
                    )

                    # residual streams in while attention runs, split queues
                    xres = bigpool.tile([128, CB, NH], f32, tag="xres")
                    for cc in range(CB):
                        eng = nc.sync if cc % 2 == 0 else nc.scalar
                        eng.dma_start(out=xres[:, cc, :], in_=xres_ext[cc * 128:(cc + 1) * 128, :])

                    # K, V, and iseg0's S^T+exp per 512-pixel key segment
                    for s in range(8):
                        for ob in range(CB):  # K for key segment s
                            ps_k = mmps.tile([128, FD], f32, tag="qkvps", name="qkvps")
                            for t in range(2):
                                nc.tensor.matmul(
                                    ps_k,
                                    lhsT=wq8[:, 2 * t:2 * t + 2, ts(CB + ob, 128)],
                                    rhs=x8[:, 2 * t:2 * t + 2, ts(s, FD)],
                                    start=(t == 0), stop=(t == 1), perf_mode=DR,
                                )
                            nc.vector.tensor_scalar(
                                out=k8_sb[:, ob, ts(s, FD)], in0=ps_k,
                                scalar1=SCALE, scalar2=bqs_new[:, CB + ob:CB + ob + 1],
                                op0=ALU.mult, op1=ALU.add,
                            )
                        for j4 in range(4):  # V^T for key blocks 4s..4s+3
                            jb = 4 * s + j4
                            ps_v = mmps.tile([128, FD], f32, tag="qkvps", name="qkvps")
                            for t in range(2):
                                nc.tensor.matmul(
                                    ps_v,
                                    lhsT=x8[:, 2 * t:2 * t + 2, ts(jb, 128)],
                                    rhs=wq8[:, 2 * t:2 * t + 2, 1024:1536],
                                    start=(t == 0), stop=(t == 1), perf_mode=DR,
                                )
                            nc.vector.tensor_add(vt_sb[:, jb, :], ps_v, bvt_sb)
                        for j4 in range(4):  # S^T + exp for iseg 0
                            jb = 4 * s + j4
                            ps_s = spool.tile([128, FD], f32, tag="sps", name="sps")
                            for t in range(2):
                                nc.tensor.matmul(
                                    ps_s,
                                    lhsT=k8_sb[:, 2 * t:2 * t + 2, ts(jb, 128)],
                                    rhs=q8_sb[:, 2 * t:2 * t + 2, ts(0, FD)],
                                    start=(t == 0), stop=(t == 1), perf_mode=DR,
                                )
                            nc.scalar.activation(
                                out=p8_0[:, jb, :], in_=ps_s,
                                func=AF.Exp, bias=nbias_sb, scale=1.0 / 16.0,
                            )
                xphase.__exit__(None, None, None)

                # ========== phase 3: attention isegs + proj/residual ==========
                with (
                    tc.tile_pool(name="attn", bufs=2) as apool,
                    tc.tile_pool(name="fin", bufs=3) as fpool,
                    tc.tile_pool(name="opsum", bufs=1, space="PSUM") as opool,
                    tc.tile_pool(name="lpsum", bufs=1, space="PSUM") as lpool,
                ):
                    def proj(o8, iseg, ob):
                        ps_p = spool.tile([128, FD], f32, tag="sps", name="sps")
                        for t in range(2):
                            nc.tensor.matmul(
                                ps_p,
                                lhsT=wp8[:, 2 * t:2 * t + 2, ts(ob, 128)],
                                rhs=o8[:, 2 * t:2 * t + 2, :],
                                start=(t == 0), stop=(t == 1), perf_mode=DR,
                            )
                        y_sb = fpool.tile([128, FD], f32, tag="y")
                        nc.vector.scalar_tensor_tensor(
                            out=y_sb, in0=ps_p, scalar=1.0 / 32.0,
                            in1=xres[:, ob, ts(iseg, FD)],
                            op0=ALU.mult, op1=ALU.add,
                        )
                        eng = nc.sync if ob % 2 == 0 else nc.scalar
                        eng.dma_start(
                            out=out_ext[ob * 128:(ob + 1) * 128, ts(iseg, FD)],
                            in_=y_sb,
                        )

                    pending = None  # (o8, iseg) whose proj is owed
                    for iseg in range(NSEG):
                        p8 = p8_0 if iseg == 0 else ppool.tile([128, JB, FD], fp8, tag="p8")
                        ps_o = opool.tile([128, CB, FD], f32)
                        lps = lpool.tile([128, FD], f32, tag="lps", name="lps")

                        def s_pair(m):
                            # S^T + exp for key blocks 2m, 2m+1 of this iseg
                            for jb in (2 * m, 2 * m + 1):
                                ps_s = spool.tile([128, FD], f32, tag="sps", name="sps")
                                for t in range(2):
                                    nc.tensor.matmul(
                                        ps_s,
                                        lhsT=k8_sb[:, 2 * t:2 * t + 2, ts(jb, 128)],
                                        rhs=q8_sb[:, 2 * t:2 * t + 2, ts(iseg, FD)],
                                        start=(t == 0), stop=(t == 1), perf_mode=DR,
                                    )
                                nc.scalar.activation(
                                    out=p8[:, jb, :], in_=ps_s,
                                    func=AF.Exp, bias=nbias_sb, scale=1.0 / 16.0,
                                )

                        if iseg > 0:
                            s_pair(0)
                        for m in range(16):
                            if iseg > 0 and m < 15:
                                s_pair(m + 1)
                            if pending is not None and 1 <= m <= 4:
                                proj(pending[0], pending[1], m - 1)
                                if m == 4:
                                    pending = None
                            # l first: its last pass starts the 1/l chain early
                            nc.tensor.matmul(
                                lps,
                                lhsT=ones128,
                                rhs=p8[:, 2 * m:2 * m + 2, :],
                                start=(m == 0), stop=(m == 15), perf_mode=DR,
                            )
                            for cb in range(CB):
                                nc.tensor.matmul(
                                    ps_o[:, cb, :],
                                    lhsT=vt_sb[:, 2 * m:2 * m + 2, ts(cb, 128)],
                                    rhs=p8[:, 2 * m:2 * m + 2, :],
                                    start=(m == 0), stop=(m == 15), perf_mode=DR,
                                )

                        # 1/l (replicated on every partition already)
                        rb_sb = apool.tile([128, FD], f32, tag="rb")
                        nc.vector.reciprocal_approx_fast(out=rb_sb, in_=lps)
                        o8 = apool.tile([128, CB, FD], fp8, tag="o8")
                        for cb in range(CB):
                            nc.vector.tensor_mul(o8[:, cb, :], ps_o[:, cb, :], rb_sb)
                        pending = (o8, iseg)

                    for ob in range(CB):  # last segment's proj
                        proj(pending[0], pending[1], ob)
            sphase.__exit__(None, None, None)
            p8phase.__exit__(None, None, None)

    return nc


def _get_nc(finalized: bool):
    key = ("nc", finalized)
    if key not in _CACHE:
        nc = build_bass()
        if finalized:
            nc.finalize()
        _CACHE[key] = nc
    return _CACHE[key]


def make_in_maps(x, gamma, beta, w_qkv, b_qkv, w_proj, b_proj):
    import ml_dtypes

    bf = ml_dtypes.bfloat16
    f8 = ml_dtypes.float8_e4m3fn
    wqkvT = np.ascontiguousarray(np.asarray(w_qkv, dtype=np.float32).T).astype(bf)
    wprojT = np.ascontiguousarray(np.asarray(w_proj, dtype=np.float32).T).astype(bf)
    # group-indicator matmul operands (group g = channels 16g..16g+15;
    # within a 128-channel block, local group = partition//16)
    p_idx = np.arange(128)
    gind = np.where((p_idx[:, None] // 16) == np.arange(8)[None, :], 1.0 / 16.0, 0.0).astype(np.float32)
    gindT = np.ascontiguousarray(np.where(
        (np.arange(8)[:, None]) == (p_idx[None, :] // 16), 1.0, 0.0).astype(np.float32))
    bq = np.asarray(b_qkv, dtype=np.float32)
    bqs = np.ascontiguousarray(4.0 * SCALE * bq[0:1024])
    bv4 = np.ascontiguousarray(4.0 * bq[1024:1536])
    bp = np.asarray(b_proj, dtype=np.float32)
    in_maps = []
    for core in range(8):
        bb, half = core // 2, core % 2
        xp = np.ascontiguousarray(x[bb].reshape(C, N)).astype(np.float32)
        if half:
            xp = np.ascontiguousarray(np.concatenate([xp[:, NH:], xp[:, :NH]], axis=1))
        in_maps.append(
            {
                "x8": (0.5 * xp).astype(f8),
                "gind": gind,
                "gindT": gindT,
                "bqs": bqs,
                "bv4": bv4,
                # proj bias pre-folded into the residual
                "xres": np.ascontiguousarray(xp[:, :NH] + bp[:, None]),
                "gamma": np.ascontiguousarray(gamma, dtype=np.float32),
                "beta": np.ascontiguousarray(beta, dtype=np.float32),
                "wqkvT": wqkvT,
                "wprojT": wprojT,
            }
        )
    return in_maps


def assemble_out(results, x_dtype=np.float32):
    b = 4
    out = np.zeros((b, C, N), dtype=np.float32)
    for core in range(8):
        bb, half = core // 2, core % 2
        out[bb, :, half * NH:(half + 1) * NH] = results[core]["out"]
    return out.reshape(b, C, 64, 64).astype(x_dtype)


def kernel(x, gamma, beta, w_qkv, b_qkv, w_proj, b_proj):
    from concourse.bass_utils import run_bass_kernel_spmd

    nc = _get_nc(finalized=True)
    in_maps = make_in_maps(x, gamma, beta, w_qkv, b_qkv, w_proj, b_proj)
    res = run_bass_kernel_spmd(nc, in_maps, core_ids=list(range(8)))
    return assemble_out(res.results, np.asarray(x).dtype)


# revision 50
# speedup vs baseline: 1.0424x; 1.0424x over previous
"""Trainium2 Bass kernel for nn_AttentionBlock (GroupNorm -> 1x1 qkv -> full
N^2 attention -> 1x1 proj -> residual) on x:(4, 512, 64, 64).

Sharding: 8 cores = (batch, query-half) pairs. Each core gets one batch's
full image (512 x 4096 pixels) with pixels rotated so that its query half is
always pixels [0:2048]; softmax/attention are permutation-invariant in the
key axis, so every core runs the identical SPMD graph with no collectives.

GroupNorm is folded into the qkv matmul: xn = sc*x + bs per channel, so
qkv = (W*diag(sc)) x + (b + W bs). The host ships x as fp8 (0.5*x) next to
the bf16 stats copy; sc rides the existing bf16->fp8 weight cast as a
per-partition activation scale, and the bias correction W bs comes from two
tiny DoubleRow matvecs (bs/sc column for q/k blocks; a 1-row x W product,
partition-broadcast by a ones matmul, for V^T). This takes the 16K-elem xn
pass off the critical path and lets qkv start right after the group stats.

Attention runs transpose-free in a key-on-partitions layout: S^T = K^T Q per
128-key block (contraction over channels), exp'd in place to fp8 P^T tiles;
O = sum_j V^T^T P^T accumulates over key blocks, so O lands as [c-part,
i-free] -- exactly what proj wants. The softmax denominator rides a
128-identical-columns ones DoubleRow matmul (l replicated across all
partitions), and 1/l comes from one fast-approx DVE reciprocal, folded into
the PSUM->SBUF eviction of O. proj psums triple-buffer through the S^T
PSUM pool and each segment's proj is interleaved into the next segment's
tensor stream so evictions never stall the queue.

All big matmuls run fp8e4 DoubleRow with fp32 PSUM. Scaling: x8 = 0.5*x,
w8 = 8*sc*w, q8/k8 = 4*c^-0.25 * (q/k), vt = 4*V^T, P8 = exp(S - 2.5)
(no max subtraction; |S| <= ~8), o8 = 4*O. The group stats run directly on
the fp8 x (split VectorE bn_stats / ScalarE accum across channel blocks;
two HWDGE queues split the input stream), and the group reduce/broadcast
is two tiny indicator matmuls instead of transposes.

HW-measured: 228.0 us per NEFF (baseline 276.9 us), rel err 1.8e-3.
"""

import os
import numpy as np

C = 512
CB = 4            # 128-channel blocks
N = 4096          # pixels per image
NH = 2048         # query pixels per core
G = 32            # groups
EPS = 1e-6
SCALE = float(C) ** -0.25
FD = 512          # psum free width
NSEG = NH // FD   # query segments per core (4)
JB = N // 128     # key blocks (32)

_CACHE = {}


def build_bass():
    import concourse.bass as bass
    import concourse.mybir as mybir
    import concourse.tile as tile
    from concourse import bacc
    from concourse.bass import ts
    f32 = mybir.dt.float32
    bf16 = mybir.dt.bfloat16
    fp8 = mybir.dt.float8e4
    AF = mybir.ActivationFunctionType
    ALU = mybir.AluOpType
    AX = mybir.AxisListType
    DR = mybir.MatmulPerfMode.DoubleRow

    nc = bacc.Bacc(None)
    x8_ext = nc.declare_dram_parameter("x8", [C, N], fp8, isOutput=False)
    gind_ext = nc.declare_dram_parameter("gind", [128, 8], f32, isOutput=False)
    gindT_ext = nc.declare_dram_parameter("gindT", [8, 128], f32, isOutput=False)
    bqs_ext = nc.declare_dram_parameter("bqs", [1024], f32, isOutput=False)
    bv4_ext = nc.declare_dram_parameter("bv4", [C], f32, isOutput=False)
    # note: bqkv/bproj reach the device only in folded form (bqs, bv4, xres)
    xres_ext = nc.declare_dram_parameter("xres", [C, NH], f32, isOutput=False)
    gamma_ext = nc.declare_dram_parameter("gamma", [C], f32, isOutput=False)
    beta_ext = nc.declare_dram_parameter("beta", [C], f32, isOutput=False)
    wqkvT_ext = nc.declare_dram_parameter("wqkvT", [C, 3 * C], bf16, isOutput=False)
    wprojT_ext = nc.declare_dram_parameter("wprojT", [C, C], bf16, isOutput=False)
    out_ext = nc.declare_dram_parameter("out", [C, NH], f32, isOutput=True)

    with tile.TileContext(nc) as tc:
        with (
            tc.tile_pool(name="const", bufs=1) as cpool,
            tc.tile_pool(name="big", bufs=1) as bigpool,
        ):
            # pools entered before xphase so they outlive it (LIFO release)
            p8phase = tc.tile_pool(name="p8", bufs=2)
            ppool = p8phase.__enter__()
            sphase = tc.tile_pool(name="spsum", bufs=3, space="PSUM")
            spool = sphase.__enter__()

            # x streams in as fp8 only (group stats run on it too -- the
            # quantization noise washes out over 64K elements/group). The two
            # HWDGE queues split the load: sync takes VectorE's stats blocks
            # (0,1) + qkv weights, scalar takes ScalarE's blocks (3,2) + wp.
            xphase = tc.tile_pool(name="xph", bufs=1)
            xpool = xphase.__enter__()
            x8 = xpool.tile([128, CB, N], fp8)
            for cc, hh in ((0, 0), (0, 1), (1, 0), (1, 1)):
                nc.sync.dma_start(
                    out=x8[:, cc, ts(hh, NH)],
                    in_=x8_ext[cc * 128:(cc + 1) * 128, ts(hh, NH)],
                )
            for cc, hh in ((3, 0), (3, 1), (2, 0), (2, 1)):
                nc.scalar.dma_start(
                    out=x8[:, cc, ts(hh, NH)],
                    in_=x8_ext[cc * 128:(cc + 1) * 128, ts(hh, NH)],
                )

            # ---- constants / weights ----
            # group indicator matmul operands: gind[p,g]=1/(16N) for
            # p//16==g (group-sum as one matmul), gindT[g,p]=1 indicator
            # (partition-broadcast of group values as one matmul)
            # small consts ride the gpsimd software queue so the two HW
            # queues stay clear for x8/weights (the rearranged bias loads
            # are descriptor-heavy)
            gind_sb = cpool.tile([128, 8], f32)
            nc.gpsimd.dma_start(out=gind_sb, in_=gind_ext[:, :])
            gindT_sb = cpool.tile([8, 128], f32)
            nc.gpsimd.dma_start(out=gindT_sb, in_=gindT_ext[:, :])

            gb_sb = cpool.tile([128, 2, CB], f32)  # gamma, beta as (p, t)
            nc.gpsimd.dma_start(out=gb_sb[:, 0, :], in_=gamma_ext.rearrange("(t p) -> p t", p=128))
            nc.gpsimd.dma_start(out=gb_sb[:, 1, :], in_=beta_ext.rearrange("(t p) -> p t", p=128))

            # q,k bias blocks pre-scaled by 4*SCALE on the host
            bqs_sb = cpool.tile([128, 8], f32)
            nc.gpsimd.dma_start(out=bqs_sb, in_=bqs_ext.rearrange("(t p) -> p t", p=128))

            # (b_proj is folded into xres on the host)
            # 4*b_v (host-scaled) broadcast along partitions: (128, 512)
            bvt_sb = cpool.tile([128, FD], f32)
            bv_slice = bv4_ext[:]
            bv_bcast = bass.AP(
                tensor=bv_slice.tensor,
                offset=bv_slice.offset,
                ap=[[0, 128]] + [list(p) for p in bv_slice.ap],
            )
            nc.gpsimd.dma_start(out=bvt_sb, in_=bv_bcast)

            eps_sb = cpool.tile([128, 1], f32)
            nc.vector.memset(eps_sb, EPS)
            nbias_sb = cpool.tile([128, 1], f32)  # global exp bias
            nc.vector.memset(nbias_sb, -2.5)
            warm_sb = cpool.tile([128, 1], f32)
            # DR all-ones stationary, 128 identical columns -> l-sum lands on
            # every partition (no separate broadcast needed)
            ones128 = cpool.tile([128, 2, 128], fp8)
            nc.vector.memset(ones128, 1.0)
            ones32 = cpool.tile([1, 128], f32)    # 1-row ones for V-bias bcast
            nc.vector.memset(ones32, 1.0)

            wqbf = cpool.tile([128, CB, 3 * C], bf16)
            nc.sync.dma_start(out=wqbf, in_=wqkvT_ext.rearrange("(t p) o -> p t o", p=128))
            wq8 = cpool.tile([128, CB, 3 * C], fp8)   # 8*sc*W, cast after stats
            wp_sb = cpool.tile([128, CB, C], bf16)
            nc.scalar.dma_start(out=wp_sb, in_=wprojT_ext.rearrange("(t p) o -> p t o", p=128))
            wp8 = cpool.tile([128, CB, C], fp8)

            # ---- persistent activations ----
            k8_sb = bigpool.tile([128, CB, N], fp8)
            vt_sb = bigpool.tile([128, JB, FD], fp8)   # 4*V^T
            q8_sb = bigpool.tile([128, CB, NH], fp8)

            # ===== phase 1: groupnorm stats on fp8 x (= 0.5*x). stat2 keeps
            # the raw fp8 moments (mean8, meansq8); the x2/x4 rescale folds
            # into the tiny group-level math. VectorE: blocks 0,1,2 via
            # bn_stats; ScalarE: block 3 via Identity/Square accum. =====
            with tc.tile_pool(name="pst", bufs=2, space="PSUM") as pst:
                stat2 = xpool.tile([128, CB, 2], f32)  # (mean8, meansq8) per channel
                st_stats = xpool.tile([128, 3, 8, 6], f32)
                mv_t = xpool.tile([128, 3, 2], f32)
                sc_scratch = xpool.tile([128, 2048], bf16)
                acc_part = xpool.tile([128, 2, 2], f32)  # block 3 halves
                for s in range(2):
                    nc.scalar.activation(
                        out=sc_scratch, in_=x8[:, 3, ts(s, 2048)],
                        func=AF.Identity, bias=0.0, scale=1.0,
                        accum_out=acc_part[:, 0, s:s + 1],
                    )
                    nc.scalar.activation(
                        out=sc_scratch, in_=x8[:, 3, ts(s, 2048)],
                        func=AF.Square, bias=0.0, scale=1.0,
                        accum_out=acc_part[:, 1, s:s + 1],
                    )
                # warm the Sqrt table now; the rstd sqrt hits it without a load
                nc.scalar.activation(out=warm_sb, in_=eps_sb, func=AF.Sqrt, bias=0.0, scale=1.0)
                for cc in range(3):
                    for s in range(8):
                        nc.vector.bn_stats(out=st_stats[:, cc, s, :], in_=x8[:, cc, ts(s, 512)])
                    nc.vector.bn_aggr(out=mv_t[:, cc, :], in_=st_stats[:, cc])
                    # mean8 ; meansq8 = var8 + mean8^2
                    nc.vector.tensor_copy(stat2[:, cc, 0:1], mv_t[:, cc, 0:1])
                    nc.vector.tensor_mul(stat2[:, cc, 1:2], mv_t[:, cc, 0:1], mv_t[:, cc, 0:1])
                    nc.vector.tensor_add(stat2[:, cc, 1:2], stat2[:, cc, 1:2], mv_t[:, cc, 1:2])
                for f in range(2):
                    nc.vector.tensor_reduce(
                        out=stat2[:, 3, f:f + 1], in_=acc_part[:, f, :],
                        axis=AX.X, op=ALU.add,
                    )
                    nc.vector.tensor_scalar_mul(stat2[:, 3, f:f + 1], stat2[:, 3, f:f + 1], 1.0 / float(N))

                # group aggregation as two tiny matmuls: gind^T @ stat2 =
                # per-group fp8 moments; gindT^T @ vals broadcasts the
                # (mean_x, rstd) pair back to every channel partition
                gs_ps = pst.tile([8, 8], f32)
                nc.tensor.matmul(gs_ps, lhsT=gind_sb, rhs=stat2[:, :, :])
                vals = xpool.tile([8, 2, CB], f32)  # (g, {mean_x, rstd}, cc)
                gsv = gs_ps.rearrange("g (cc f) -> g cc f", f=2)
                var_g = xpool.tile([8, CB], f32)
                nc.vector.tensor_scalar_mul(vals[:, 0, :], gsv[:, :, 0], 2.0)
                nc.vector.tensor_scalar_mul(var_g, gsv[:, :, 1], 4.0)
                nc.vector.tensor_mul(vals[:, 1, :], vals[:, 0, :], vals[:, 0, :])
                nc.vector.tensor_tensor(var_g, var_g, vals[:, 1, :], ALU.subtract)
                nc.scalar.activation(out=var_g, in_=var_g, func=AF.Sqrt, bias=eps_sb[:8], scale=1.0)
                nc.vector.reciprocal(vals[:, 1, :], var_g)
                mr_ps = pst.tile([128, 8], f32)
                nc.tensor.matmul(mr_ps, lhsT=gindT_sb, rhs=vals[:, :, :])

                # per-channel xn = sc*x + bs; sc folds into the weight cast,
                # bs into bias-correction matvecs (bsc8 = 16*bs/sc column)
                sc_sb = xpool.tile([128, CB], f32)
                bs_sb = xpool.tile([128, CB], f32)
                tmp_c = xpool.tile([128, CB], f32)
                nc.vector.tensor_mul(sc_sb, gb_sb[:, 0, :], mr_ps[:, 4:8])
                nc.vector.tensor_mul(tmp_c, mr_ps[:, 0:4], sc_sb)
                nc.vector.tensor_tensor(bs_sb, gb_sb[:, 1, :], tmp_c, ALU.subtract)
                scx8 = xpool.tile([128, CB], f32)
                nc.vector.tensor_scalar_mul(scx8, sc_sb, 8.0)
                rsc = xpool.tile([128, CB], f32)
                nc.vector.reciprocal(rsc, sc_sb)
                bsc = xpool.tile([128, CB], f32)
                nc.vector.tensor_mul(bsc, bs_sb, rsc)
                bsc8 = xpool.tile([128, CB, 16], fp8)  # col 0; 16B DR pair step
                nc.vector.tensor_scalar_mul(bsc8[:, :, 0:1], bsc[:, :, None], 16.0)

                # W' = 8*sc*W cast bf16->fp8, q cols first so Q starts ASAP;
                # split scalar/vector by cc block
                for cols in range(3):
                    for cc in range(CB):
                        src = wqbf[:, cc, ts(cols, C)]
                        dst = wq8[:, cc, ts(cols, C)]
                        if cc % 2 == 0:
                            nc.scalar.activation(
                                out=dst, in_=src, func=AF.Copy, bias=0.0,
                                scale=scx8[:, cc:cc + 1],
                            )
                        else:
                            nc.vector.tensor_scalar_mul(dst, src, scx8[:, cc:cc + 1])
                # warm the Exp table now so the first attention exp doesn't
                # pay the table load (Copy casts don't touch the table)
                nc.scalar.activation(out=warm_sb, in_=eps_sb, func=AF.Exp, bias=0.0, scale=1.0)

            # ====== phase 2: qkv projections fused with iseg0 S^T+exp ======
            if True:
                p8_0 = ppool.tile([128, JB, FD], fp8, tag="p8")

                with tc.tile_pool(name="mmps", bufs=5, space="PSUM") as mmps:
                    # q/k bias corrections: bias_ps[:, blk] = 128*(W bs)[blk]
                    bias_ps = mmps.tile([128, FD], f32, tag="qkvps", name="qkvps")
                    bqs_new = xpool.tile([128, 8], f32)
                    for half in range(2):  # q blocks 0-3, k blocks 4-7
                        for b4 in range(CB):
                            blk = 4 * half + b4
                            for t in range(2):
                                nc.tensor.matmul(
                                    bias_ps[:, blk:blk + 1],
                                    lhsT=wq8[:, 2 * t:2 * t + 2, ts(blk, 128)],
                                    rhs=bsc8[:, 2 * t:2 * t + 2, 0:1],
                                    start=(t == 0), stop=(t == 1), perf_mode=DR,
                                )
                        nc.vector.scalar_tensor_tensor(
                            out=bqs_new[:, 4 * half:4 * half + 4],
                            in0=bias_ps[:, 4 * half:4 * half + 4],
                            scalar=SCALE / 32.0,
                            in1=bqs_sb[:, 4 * half:4 * half + 4],
                            op0=ALU.mult, op1=ALU.add,
                        )

                    for ob in range(CB):  # Q, first NH pixels
                        pss = [mmps.tile([128, FD], f32, tag="qkvps", name="qkvps") for _ in range(NSEG)]
                        for t in range(2):
                            for iseg in range(NSEG):
                                nc.tensor.matmul(
                                    pss[iseg],
                                    lhsT=wq8[:, 2 * t:2 * t + 2, ts(ob, 128)],
                                    rhs=x8[:, 2 * t:2 * t + 2, ts(iseg, FD)],
                                    start=(t == 0), stop=(t == 1), perf_mode=DR,
                                )
                        for iseg in range(NSEG):
                            nc.vector.tensor_scalar(
                                out=q8_sb[:, ob, ts(iseg, FD)], in0=pss[iseg],
                                scalar1=SCALE, scalar2=bqs_new[:, ob:ob + 1],
                                op0=ALU.mult, op1=ALU.add,
                            )

                    # V bias correction row: 128*(W_v bs) as [1, 512], then
                    # partition-broadcast via ones32 matmul, folded into bvt
                    ps_vr = mmps.tile([128, FD], f32, tag="qkvps", name="qkvps")
                    for t in range(2):
                        nc.tensor.matmul(
                            ps_vr[0:1, :],
                            lhsT=bsc8[:, 2 * t:2 * t + 2, 0:1],
                            rhs=wq8[:, 2 * t:2 * t + 2, 1024:1536],
                            start=(t == 0), stop=(t == 1), perf_mode=DR,
                        )
                    vrow_sb = xpool.tile([1, FD], f32)
                    nc.scalar.activation(out=vrow_sb, in_=ps_vr[0:1, :], func=AF.Copy, bias=0.0, scale=1.0)
                    ps_vb = mmps.tile([128, FD], f32, tag="qkvps", name="qkvps")
                    nc.tensor.matmul(ps_vb, lhsT=ones32, rhs=vrow_sb)
                    nc.vector.scalar_tensor_tensor(
                        out=bvt_sb, in0=ps_vb, scalar=1.0 / 32.0, in1=bvt_sb,
                        op0=ALU.mult, op1=ALU.add,
                    )

                    # residual streams in while attention runs, split queues
                    xres = bigpool.tile([128, CB, NH], f32, tag="xres")
                    for cc in range(CB):
                        eng = nc.sync if cc % 2 == 0 else nc.scalar
                        eng.dma_start(out=xres[:, cc, :], in_=xres_ext[cc * 128:(cc + 1) * 128, :])

                    # K, V, and iseg0's S^T+exp per 512-pixel key segment
                    for s in range(8):
                        for ob in range(CB):  # K for key segment s
                            ps_k = mmps.tile([128, FD], f32, tag="qkvps", name="qkvps")
                            for t in range(2):
                                nc.tensor.matmul(
                                    ps_k,
                                    lhsT=wq8[:, 2 * t:2 * t + 2, ts(CB + ob, 128)],
                                    rhs=x8[:, 2 * t:2 * t + 2, ts(s, FD)],
                                    start=(t == 0), stop=(t == 1), perf_mode=DR,
                                )
                            nc.vector.tensor_scalar(
                                out=k8_sb[:, ob, ts(s, FD)], in0=ps_k,
                                scalar1=SCALE, scalar2=bqs_new[:, CB + ob:CB + ob + 1],
                                op0=ALU.mult, op1=ALU.add,
                            )
                        for j4 in range(4):  # V^T for key blocks 4s..4s+3
                            jb = 4 * s + j4
                            ps_v = mmps.tile([128, FD], f32, tag="qkvps", name="qkvps")
                            for t in range(2):
                                nc.tensor.matmul(
                                    ps_v,
                                    lhsT=x8[:, 2 * t:2 * t + 2, ts(jb, 128)],
                                    rhs=wq8[:, 2 * t:2 * t + 2, 1024:1536],
                                    start=(t == 0), stop=(t == 1), perf_mode=DR,
                                )
                            nc.vector.tensor_add(vt_sb[:, jb, :], ps_v, bvt_sb)
                        for j4 in range(4):  # S^T + exp for iseg 0
                            jb = 4 * s + j4
                            ps_s = spool.tile([128, FD], f32, tag="sps", name="sps")
                            for t in range(2):
                                nc.tensor.matmul(
                                    ps_s,
                                    lhsT=k8_sb[:, 2 * t:2 * t + 2, ts(jb, 128)],
                                    rhs=q8_sb[:, 2 * t:2 * t + 2, ts(0, FD)],
                                    start=(t == 0), stop=(t == 1), perf_mode=DR,
                                )
                            nc.scalar.activation(
                                out=p8_0[:, jb, :], in_=ps_s,
                                func=AF.Exp, bias=nbias_sb, scale=1.0 / 16.0,
                            )
                xphase.__exit__(None, None, None)

                # ========== phase 3: attention isegs + proj/residual ==========
                with (
                    tc.tile_pool(name="attn", bufs=2) as apool,
                    tc.tile_pool(name="fin", bufs=3) as fpool,
                    tc.tile_pool(name="opsum", bufs=1, space="PSUM") as opool,
                    tc.tile_pool(name="lpsum", bufs=1, space="PSUM") as lpool,
                ):
                    # wp8 cast here: ScalarE is idle now and the pre-attention
                    # queue stays clear for the first exps
                    nc.scalar.activation(out=wp8, in_=wp_sb, func=AF.Copy, bias=0.0, scale=8.0)

                    def proj(o8, iseg, ob):
                        ps_p = spool.tile([128, FD], f32, tag="sps", name="sps")
                        for t in range(2):
                            nc.tensor.matmul(
                                ps_p,
                                lhsT=wp8[:, 2 * t:2 * t + 2, ts(ob, 128)],
                                rhs=o8[:, 2 * t:2 * t + 2, :],
                                start=(t == 0), stop=(t == 1), perf_mode=DR,
                            )
                        y_sb = fpool.tile([128, FD], f32, tag="y")
                        nc.vector.scalar_tensor_tensor(
                            out=y_sb, in0=ps_p, scalar=1.0 / 32.0,
                            in1=xres[:, ob, ts(iseg, FD)],
                            op0=ALU.mult, op1=ALU.add,
                        )
                        eng = nc.sync if ob % 2 == 0 else nc.scalar
                        eng.dma_start(
                            out=out_ext[ob * 128:(ob + 1) * 128, ts(iseg, FD)],
                            in_=y_sb,
                        )

                    pending = None  # (o8, iseg) whose proj is owed
                    for iseg in range(NSEG):
                        p8 = p8_0 if iseg == 0 else ppool.tile([128, JB, FD], fp8, tag="p8")
                        ps_o = opool.tile([128, CB, FD], f32)
                        lps = lpool.tile([128, FD], f32, tag="lps", name="lps")

                        def s_pair(m):
                            # S^T + exp for key blocks 2m, 2m+1 of this iseg
                            for jb in (2 * m, 2 * m + 1):
                                ps_s = spool.tile([128, FD], f32, tag="sps", name="sps")
                                for t in range(2):
                                    nc.tensor.matmul(
                                        ps_s,
                                        lhsT=k8_sb[:, 2 * t:2 * t + 2, ts(jb, 128)],
                                        rhs=q8_sb[:, 2 * t:2 * t + 2, ts(iseg, FD)],
                                        start=(t == 0), stop=(t == 1), perf_mode=DR,
                                    )
                                nc.scalar.activation(
                                    out=p8[:, jb, :], in_=ps_s,
                                    func=AF.Exp, bias=nbias_sb, scale=1.0 / 16.0,
                                )

                        if iseg > 0:
                            s_pair(0)
                        for m in range(16):
                            if iseg > 0 and m < 15:
                                s_pair(m + 1)
                            if pending is not None and 1 <= m <= 4:
                                proj(pending[0], pending[1], m - 1)
                                if m == 4:
                                    pending = None
                            # l first: its last pass starts the 1/l chain early
                            nc.tensor.matmul(
                                lps,
                                lhsT=ones128,
                                rhs=p8[:, 2 * m:2 * m + 2, :],
                                start=(m == 0), stop=(m == 15), perf_mode=DR,
                            )
                            for cb in range(CB):
                                nc.tensor.matmul(
                                    ps_o[:, cb, :],
                                    lhsT=vt_sb[:, 2 * m:2 * m + 2, ts(cb, 128)],
                                    rhs=p8[:, 2 * m:2 * m + 2, :],
                                    start=(m == 0), stop=(m == 15), perf_mode=DR,
                                )

                        # 1/l (replicated on every partition already)
                        rb_sb = apool.tile([128, FD], f32, tag="rb")
                        nc.vector.reciprocal_approx_fast(out=rb_sb, in_=lps)
                        o8 = apool.tile([128, CB, FD], fp8, tag="o8")
                        if iseg < NSEG - 1:
                            for cb in range(CB):
                                nc.vector.tensor_mul(o8[:, cb, :], ps_o[:, cb, :], rb_sb)
                            pending = (o8, iseg)
                        else:
                            # last segment: normalize and project in 256-px
                            # halves so the tail chain pipelines
                            for h in range(2):
                                hs = ts(h, 256)
                                for cb in range(CB):
                                    nc.vector.tensor_mul(
                                        o8[:, cb, hs], ps_o[:, cb, hs], rb_sb[:, hs])
                                for ob in range(CB):
                                    ps_p = spool.tile([128, FD], f32, tag="sps", name="sps")
                                    for t in range(2):
                                        nc.tensor.matmul(
                                            ps_p[:, 0:256],
                                            lhsT=wp8[:, 2 * t:2 * t + 2, ts(ob, 128)],
                                            rhs=o8[:, 2 * t:2 * t + 2, hs],
                                            start=(t == 0), stop=(t == 1), perf_mode=DR,
                                        )
                                    y_sb = fpool.tile([128, 256], f32, tag="yh")
                                    nc.vector.scalar_tensor_tensor(
                                        out=y_sb, in0=ps_p[:, 0:256], scalar=1.0 / 32.0,
                                        in1=xres[:, ob, ts(2 * iseg + h, 256)],
                                        op0=ALU.mult, op1=ALU.add,
                                    )
                                    eng = nc.sync if ob % 2 == 0 else nc.scalar
                                    eng.dma_start(
                                        out=out_ext[ob * 128:(ob + 1) * 128,
                                                    ts(2 * iseg + h, 256)],
                                        in_=y_sb,
                                    )
            sphase.__exit__(None, None, None)
            p8phase.__exit__(None, None, None)

    return nc


def _get_nc(finalized: bool):
    key = ("nc", finalized)
    if key not in _CACHE:
        nc = build_bass()
        if finalized:
            nc.finalize()
        _CACHE[key] = nc
    return _CACHE[key]


def make_in_maps(x, gamma, beta, w_qkv, b_qkv, w_proj, b_proj):
    import ml_dtypes

    bf = ml_dtypes.bfloat16
    f8 = ml_dtypes.float8_e4m3fn
    wqkvT = np.ascontiguousarray(np.asarray(w_qkv, dtype=np.float32).T).astype(bf)
    wprojT = np.ascontiguousarray(np.asarray(w_proj, dtype=np.float32).T).astype(bf)
    # group-indicator matmul operands (group g = channels 16g..16g+15;
    # within a 128-channel block, local group = partition//16)
    p_idx = np.arange(128)
    gind = np.where((p_idx[:, None] // 16) == np.arange(8)[None, :], 1.0 / 16.0, 0.0).astype(np.float32)
    gindT = np.ascontiguousarray(np.where(
        (np.arange(8)[:, None]) == (p_idx[None, :] // 16), 1.0, 0.0).astype(np.float32))
    bq = np.asarray(b_qkv, dtype=np.float32)
    bqs = np.ascontiguousarray(4.0 * SCALE * bq[0:1024])
    bv4 = np.ascontiguousarray(4.0 * bq[1024:1536])
    bp = np.asarray(b_proj, dtype=np.float32)
    in_maps = []
    for core in range(8):
        bb, half = core // 2, core % 2
        xp = np.ascontiguousarray(x[bb].reshape(C, N)).astype(np.float32)
        if half:
            xp = np.ascontiguousarray(np.concatenate([xp[:, NH:], xp[:, :NH]], axis=1))
        in_maps.append(
            {
                "x8": (0.5 * xp).astype(f8),
                "gind": gind,
                "gindT": gindT,
                "bqs": bqs,
                "bv4": bv4,
                # proj bias pre-folded into the residual
                "xres": np.ascontiguousarray(xp[:, :NH] + bp[:, None]),
                "gamma": np.ascontiguousarray(gamma, dtype=np.float32),
                "beta": np.ascontiguousarray(beta, dtype=np.float32),
                "wqkvT": wqkvT,
                "wprojT": wprojT,
            }
        )
    return in_maps


def assemble_out(results, x_dtype=np.float32):
    b = 4
    out = np.zeros((b, C, N), dtype=np.float32)
    for core in range(8):
        bb, half = core // 2, core % 2
        out[bb, :, half * NH:(half + 1) * NH] = results[core]["out"]
    return out.reshape(b, C, 64, 64).astype(x_dtype)


def kernel(x, gamma, beta, w_qkv, b_qkv, w_proj, b_proj):
    from concourse.bass_utils import run_bass_kernel_spmd

    nc = _get_nc(finalized=True)
    in_maps = make_in_maps(x, gamma, beta, w_qkv, b_qkv, w_proj, b_proj)
    res = run_bass_kernel_spmd(nc, in_maps, core_ids=list(range(8)))
    return assemble_out(res.results, np.asarray(x).dtype)
